# revision 1
# baseline (speedup 1.0000x reference)
"""GroupedSwiGLU MoE kernel for 8x Trainium2 NeuronCores.

Strategy: expert-parallel. Core e gets expert e's weights and its (padded)
token group. Inside each core:
  phase 1: gateT/upT[inter, tok] = Wg/Wu^T-contracted matmuls vs xT
  swiglu : hT = silu(gateT) * upT
  phase 2: out[tok, hid] = hT^T-contracted matmuls vs Wd, scaled by probs
All matmul operands bf16 (fp32 PSUM accumulate); host does the
transpose/tiling/padding and the final gather.
"""

import numpy as np
import ml_dtypes
from contextlib import ExitStack

import concourse.bass as bass
import concourse.mybir as mybir
import concourse.tile as tile
from concourse.bacc import Bacc
from concourse.bass_utils import run_bass_kernel_spmd

E = 8
HID = 2048
INTER = 1408
P = 128
KO_H = HID // P    # 16 k-tiles for phase-1 contraction
KO_I = INTER // P  # 11 k-tiles for phase-2 contraction / m-tiles in phase 1
TF = 512           # phase-1 moving free chunk (tokens)
NF = 512           # phase-2 moving free chunk (hid)

F32 = mybir.dt.float32
BF16 = mybir.dt.bfloat16
NP_BF16 = ml_dtypes.bfloat16

_nc_cache: dict = {}


def _build(T: int):
    """Per-core Bass program for T padded tokens (T % 512 == 0)."""
    nc = Bacc()
    xT = nc.dram_tensor("xT", [P, KO_H, T], BF16, kind="ExternalInput")
    wg = nc.dram_tensor("wg", [P, KO_I, KO_H, P], BF16, kind="ExternalInput")
    wu = nc.dram_tensor("wu", [P, KO_I, KO_H, P], BF16, kind="ExternalInput")
    wd = nc.dram_tensor("wd", [P, KO_I, HID], BF16, kind="ExternalInput")
    probs = nc.dram_tensor("probs", [P, T // P], F32, kind="ExternalInput")
    out = nc.dram_tensor("out", [T, HID], F32, kind="ExternalOutput")

    n_tf = T // TF
    n_t = T // P
    n_nf = HID // NF

    with tile.TileContext(nc) as tc, ExitStack() as ctx:
        resident = ctx.enter_context(tc.tile_pool(name="resident", bufs=1))
        wpool = ctx.enter_context(tc.tile_pool(name="weights", bufs=2))
        tmp = ctx.enter_context(tc.tile_pool(name="tmp", bufs=3))
        opool = ctx.enter_context(tc.tile_pool(name="outp", bufs=4))
        psum = ctx.enter_context(tc.tile_pool(name="psum", bufs=2, space="PSUM"))

        xT_sb = resident.tile([P, KO_H, T], BF16)
        for k in range(KO_H):
            nc.sync.dma_start(xT_sb[:, k], xT[:, k])
        wd_sb = resident.tile([P, KO_I, HID], BF16)
        for k in range(KO_I):
            nc.sync.dma_start(wd_sb[:, k], wd[:, k])
        probs_dma = resident.tile([P, T // P], F32)
        nc.sync.dma_start(probs_dma[:], probs[:])
        # Bounce through DVE so phase-2 scaling (DVE) only ever needs the PE
        # wait: the TensorScalar ISA slot can't carry a second (DMA) wait.
        probs_sb = resident.tile([P, T // P], F32)
        nc.vector.tensor_copy(probs_sb[:], probs_dma[:])
        hT_sb = resident.tile([P, KO_I, T], BF16)

        # Phase 1: for each inter m-tile, gateT/upT psum then fused silu*mul
        for m in range(KO_I):
            wg_m = wpool.tile([P, KO_H, P], BF16, tag="wg")
            nc.gpsimd.dma_start(wg_m[:], wg[:, m])
            wu_m = wpool.tile([P, KO_H, P], BF16, tag="wu")
            nc.gpsimd.dma_start(wu_m[:], wu[:, m])
            for f in range(n_tf):
                pg = psum.tile([P, TF], F32, tag="pg")
                pu = psum.tile([P, TF], F32, tag="pu")
                for k in range(KO_H):
                    nc.tensor.matmul(
                        pg[:], wg_m[:, k], xT_sb[:, k, bass.ts(f, TF)],
                        start=(k == 0), stop=(k == KO_H - 1),
                    )
                for k in range(KO_H):
                    nc.tensor.matmul(
                        pu[:], wu_m[:, k], xT_sb[:, k, bass.ts(f, TF)],
                        start=(k == 0), stop=(k == KO_H - 1),
                    )
                sg = tmp.tile([P, TF], F32, tag="sg")
                nc.scalar.activation(
                    sg[:], pg[:], mybir.ActivationFunctionType.Silu
                )
                # ACT copy of up-psum so the DVE mul has a single-engine wait
                su = tmp.tile([P, TF], F32, tag="su")
                nc.scalar.copy(su[:], pu[:])
                nc.vector.tensor_mul(
                    hT_sb[:, m, bass.ts(f, TF)], sg[:], su[:]
                )

        # Phase 2: out tiles [128 tok, 512 hid], contract over inter
        for t in range(n_t):
            for n in range(n_nf):
                po = psum.tile([P, NF], F32, tag="po")
                for k in range(KO_I):
                    nc.tensor.matmul(
                        po[:], hT_sb[:, k, bass.ts(t, P)],
                        wd_sb[:, k, bass.ts(n, NF)],
                        start=(k == 0), stop=(k == KO_I - 1),
                    )
                ot = opool.tile([P, NF], F32, tag="ot")
                nc.vector.tensor_scalar_mul(ot[:], po[:], probs_sb[:, t : t + 1])
                nc.sync.dma_start(out[bass.ts(t, P), bass.ts(n, NF)], ot[:])
    nc.finalize()
    return nc


def _pack_core(x_pad, probs_pad, wg_e, wu_e, wd_e, T):
    """Host-side tiling into the DRAM layouts the kernel expects."""
    # xT[p, k, t] = x_pad[t, k*128+p]
    xT = np.ascontiguousarray(
        x_pad.T.reshape(KO_H, P, T).transpose(1, 0, 2)
    ).astype(NP_BF16)
    # wg[p, m, k, i] = w_gate[k*128+p, m*128+i]
    wgt = np.ascontiguousarray(
        wg_e.reshape(KO_H, P, KO_I, P).transpose(1, 2, 0, 3)
    ).astype(NP_BF16)
    wut = np.ascontiguousarray(
        wu_e.reshape(KO_H, P, KO_I, P).transpose(1, 2, 0, 3)
    ).astype(NP_BF16)
    # wd[p, k, h] = w_down[k*128+p, h]
    wdt = np.ascontiguousarray(
        wd_e.reshape(KO_I, P, HID).transpose(1, 0, 2)
    ).astype(NP_BF16)
    # probs[p, o] = probs_pad[o*128+p]
    pr = np.ascontiguousarray(probs_pad.reshape(T // P, P).T).astype(np.float32)
    return {"xT": xT, "wg": wgt, "wu": wut, "wd": wdt, "probs": pr}


def _run(inputs, trace=False):
    x = np.asarray(inputs["permuted_x"], np.float32)
    probs = np.asarray(inputs["permuted_probs"], np.float32)
    wg = np.asarray(inputs["w_gate"], np.float32)
    wu = np.asarray(inputs["w_up"], np.float32)
    wd = np.asarray(inputs["w_down"], np.float32)
    counts = np.asarray(inputs["tokens_per_expert"]).astype(np.int64)
    offs = np.concatenate([[0], np.cumsum(counts)])
    assert offs[-1] == x.shape[0]

    T = int(max(1, counts.max()))
    T = ((T + 511) // 512) * 512

    key = T
    if key not in _nc_cache:
        _nc_cache[key] = _build(T)
    nc = _nc_cache[key]

    in_maps = []
    for e in range(E):
        n = int(counts[e])
        s = int(offs[e])
        x_pad = np.zeros((T, HID), np.float32)
        x_pad[:n] = x[s : s + n]
        p_pad = np.zeros((T,), np.float32)
        p_pad[:n] = probs[s : s + n]
        in_maps.append(_pack_core(x_pad, p_pad, wg[e], wu[e], wd[e], T))

    res = run_bass_kernel_spmd(nc, in_maps, core_ids=list(range(E)), trace=trace)

    y = np.empty((x.shape[0], HID), np.float32)
    for e in range(E):
        n = int(counts[e])
        s = int(offs[e])
        y[s : s + n] = res.results[e]["out"][:n]
    return y, res


def kernel(**inputs) -> np.ndarray:
    y, _ = _run(inputs, trace=False)
    return y



# revision 2
# speedup vs baseline: 1.2091x; 1.2091x over previous
"""GroupedSwiGLU MoE kernel for 8x Trainium2 NeuronCores.

Strategy: token-balanced packing. Every core gets exactly 1024 tokens as
three fixed-length segments (384, 384, 256); each segment is bound to one
expert's weights (passed as that slot's weight inputs). The given
tokens_per_expert decompose exactly into 16x384 + 8x256 chunks, so all
cores do identical FLOPs (the PE roofline) with zero padding.

Inside each core, per segment:
  phase 1: gateT/upT[inter, tok] psum chains contracting HID, then
           fused silu*mul into a resident hT buffer (bf16)
  phase 2: out[tok, hid] chains contracting INTER against the slot's
           w_down, scaled by probs
All matmul operands bf16 (fp32 PSUM accumulate); host does the
transpose/tiling and the final scatter back to original token order.

Falls back to the padded expert-parallel scheme (core e = expert e,
padded to the max group size) for inputs that don't fit the packing.
"""

import numpy as np
import ml_dtypes
from contextlib import ExitStack

import concourse.bass as bass
import concourse.mybir as mybir
import concourse.tile as tile
from concourse.bacc import Bacc
from concourse.bass_utils import run_bass_kernel_spmd

E = 8
HID = 2048
INTER = 1408
P = 128
KO_H = HID // P    # 16 k-tiles for phase-1 contraction
KO_I = INTER // P  # 11 k-tiles for phase-2 contraction / m-tiles in phase 1
NF = 512           # phase-2 moving free chunk (hid)

SEG_LENS = (384, 384, 256)  # per-core token segments (sum = 1024)
T_CORE = sum(SEG_LENS)

F32 = mybir.dt.float32
BF16 = mybir.dt.bfloat16
NP_BF16 = ml_dtypes.bfloat16

_nc_cache: dict = {}


# ---------------------------------------------------------------- packed path

def _build_packed():
    """Per-core Bass program: 1024 tokens in segments SEG_LENS, one expert
    weight set per segment slot."""
    nc = Bacc()
    xs = [
        nc.dram_tensor(f"x{j}", [P, KO_H, L], BF16, kind="ExternalInput")
        for j, L in enumerate(SEG_LENS)
    ]
    wgs = [
        nc.dram_tensor(f"wg{j}", [P, KO_I, KO_H, P], BF16, kind="ExternalInput")
        for j in range(len(SEG_LENS))
    ]
    wus = [
        nc.dram_tensor(f"wu{j}", [P, KO_I, KO_H, P], BF16, kind="ExternalInput")
        for j in range(len(SEG_LENS))
    ]
    wds = [
        nc.dram_tensor(f"wd{j}", [P, KO_I, HID], BF16, kind="ExternalInput")
        for j in range(len(SEG_LENS))
    ]
    probs = nc.dram_tensor("probs", [P, T_CORE // P], F32, kind="ExternalInput")
    out = nc.dram_tensor("out", [T_CORE, HID], F32, kind="ExternalOutput")

    lmax = max(SEG_LENS)

    with tile.TileContext(nc) as tc, ExitStack() as ctx:
        resident = ctx.enter_context(tc.tile_pool(name="resident", bufs=1))
        wpool = ctx.enter_context(tc.tile_pool(name="weights", bufs=2))
        wdpool = ctx.enter_context(tc.tile_pool(name="wdp", bufs=2))
        tmp = ctx.enter_context(tc.tile_pool(name="tmp", bufs=3))
        opool = ctx.enter_context(tc.tile_pool(name="outp", bufs=4))
        psum = ctx.enter_context(tc.tile_pool(name="psum", bufs=2, space="PSUM"))

        x_sb = []
        for j, L in enumerate(SEG_LENS):
            xt = resident.tile([P, KO_H, L], BF16)
            nc.sync.dma_start(xt[:], xs[j][:])
            x_sb.append(xt)
        probs_dma = resident.tile([P, T_CORE // P], F32)
        nc.sync.dma_start(probs_dma[:], probs[:])
        # Bounce through DVE so phase-2 scaling (DVE) only ever needs the PE
        # wait: the TensorScalar ISA slot can't carry a second (DMA) wait.
        probs_sb = resident.tile([P, T_CORE // P], F32)
        nc.vector.tensor_copy(probs_sb[:], probs_dma[:])
        hT_sb = resident.tile([P, KO_I, T_CORE], BF16)

        t0 = 0
        for j, L in enumerate(SEG_LENS):
            # w_down for this segment streams on the scalar queue during ph1
            wd_sb = wdpool.tile([P, KO_I, HID], BF16, tag="wd")
            nc.scalar.dma_start(wd_sb[:], wds[j][:])

            # Phase 1: per inter m-tile, gateT/upT psum then fused silu*mul
            for m in range(KO_I):
                wg_m = wpool.tile([P, KO_H, P], BF16, tag="wg")
                nc.gpsimd.dma_start(wg_m[:], wgs[j][:, m])
                wu_m = wpool.tile([P, KO_H, P], BF16, tag="wu")
                nc.gpsimd.dma_start(wu_m[:], wus[j][:, m])
                pg = psum.tile([P, lmax], F32, tag="pg")
                pu = psum.tile([P, lmax], F32, tag="pu")
                for k in range(KO_H):
                    nc.tensor.matmul(
                        pg[:, :L], wg_m[:, k], x_sb[j][:, k],
                        start=(k == 0), stop=(k == KO_H - 1),
                    )
                for k in range(KO_H):
                    nc.tensor.matmul(
                        pu[:, :L], wu_m[:, k], x_sb[j][:, k],
                        start=(k == 0), stop=(k == KO_H - 1),
                    )
                sg = tmp.tile([P, lmax], F32, tag="sg")
                nc.scalar.activation(
                    sg[:, :L], pg[:, :L], mybir.ActivationFunctionType.Silu
                )
                # ACT copy of up-psum so the DVE mul has a single-engine wait
                su = tmp.tile([P, lmax], F32, tag="su")
                nc.scalar.copy(su[:, :L], pu[:, :L])
                nc.vector.tensor_mul(
                    hT_sb[:, m, t0 : t0 + L], sg[:, :L], su[:, :L]
                )

            # Phase 2: out tiles [128 tok, 512 hid], contract over inter
            for tt in range(t0 // P, (t0 + L) // P):
                for n in range(HID // NF):
                    po = psum.tile([P, NF], F32, tag="po")
                    for k in range(KO_I):
                        nc.tensor.matmul(
                            po[:], hT_sb[:, k, tt * P : (tt + 1) * P],
                            wd_sb[:, k, n * NF : (n + 1) * NF],
                            start=(k == 0), stop=(k == KO_I - 1),
                        )
                    ot = opool.tile([P, NF], F32, tag="ot")
                    nc.vector.tensor_scalar_mul(
                        ot[:], po[:], probs_sb[:, tt : tt + 1]
                    )
                    nc.sync.dma_start(
                        out[tt * P : (tt + 1) * P, n * NF : (n + 1) * NF], ot[:]
                    )
            t0 += L
    nc.finalize()
    return nc


def _plan(counts):
    """Decompose expert token counts into 16x384 + 8x256 chunks and assign
    them to 8 cores x slots (384, 384, 256). Returns per-core slot lists
    [(expert, start_within_expert, length), ...] or None if infeasible."""
    n384 = 2 * E
    n256 = E
    opts_per_e = []
    for c in counts:
        opts = []
        for a in range(c // 384 + 1):
            r = c - 384 * a
            if r >= 0 and r % 256 == 0:
                opts.append((a, r // 256))
        if not opts:
            return None
        opts_per_e.append(opts)

    def rec(i, a, b, acc):
        if i == len(counts):
            return list(acc) if (a == n384 and b == n256) else None
        for ae, be in opts_per_e[i]:
            if a + ae <= n384 and b + be <= n256:
                acc.append((ae, be))
                r = rec(i + 1, a + ae, b + be, acc)
                if r is not None:
                    return r
                acc.pop()
        return None

    decomp = rec(0, 0, 0, [])
    if decomp is None:
        return None

    chunks384, chunks256 = [], []
    for e, (a, b) in enumerate(decomp):
        for i in range(a):
            chunks384.append((e, i * 384, 384))
        for i in range(b):
            chunks256.append((e, a * 384 + i * 256, 256))
    return [
        [chunks384[2 * c], chunks384[2 * c + 1], chunks256[c]] for c in range(E)
    ]


def _pack_expert(wg_e, wu_e, wd_e):
    """Tile one expert's weights into the DRAM layouts the kernel expects."""
    # wg[p, m, k, i] = w_gate[k*128+p, m*128+i]
    wgt = np.ascontiguousarray(
        wg_e.reshape(KO_H, P, KO_I, P).transpose(1, 2, 0, 3)
    ).astype(NP_BF16)
    wut = np.ascontiguousarray(
        wu_e.reshape(KO_H, P, KO_I, P).transpose(1, 2, 0, 3)
    ).astype(NP_BF16)
    # wd[p, k, h] = w_down[k*128+p, h]
    wdt = np.ascontiguousarray(
        wd_e.reshape(KO_I, P, HID).transpose(1, 0, 2)
    ).astype(NP_BF16)
    return wgt, wut, wdt


def _pack_xT(x_rows):
    """xT[p, k, t] = x_rows[t, k*128+p] for one segment."""
    L = x_rows.shape[0]
    return np.ascontiguousarray(
        x_rows.T.reshape(KO_H, P, L).transpose(1, 0, 2)
    ).astype(NP_BF16)


def _run_packed(x, probs, wg, wu, wd, counts, offs, plan, trace=False):
    if "packed" not in _nc_cache:
        _nc_cache["packed"] = _build_packed()
    nc = _nc_cache["packed"]

    packed_w = [_pack_expert(wg[e], wu[e], wd[e]) for e in range(E)]

    in_maps = []
    for c in range(E):
        m = {}
        p_core = np.empty((T_CORE,), np.float32)
        t0 = 0
        for j, (e, s, L) in enumerate(plan[c]):
            g = int(offs[e]) + s
            m[f"x{j}"] = _pack_xT(x[g : g + L])
            m[f"wg{j}"], m[f"wu{j}"], m[f"wd{j}"] = packed_w[e]
            p_core[t0 : t0 + L] = probs[g : g + L]
            t0 += L
        m["probs"] = np.ascontiguousarray(
            p_core.reshape(T_CORE // P, P).T
        ).astype(np.float32)
        in_maps.append(m)

    res = run_bass_kernel_spmd(nc, in_maps, core_ids=list(range(E)), trace=trace)

    y = np.empty((x.shape[0], HID), np.float32)
    for c in range(E):
        o = res.results[c]["out"]
        t0 = 0
        for e, s, L in plan[c]:
            g = int(offs[e]) + s
            y[g : g + L] = o[t0 : t0 + L]
            t0 += L
    return y, res


# ------------------------------------------------- padded fallback (generic)

def _build_padded(T: int):
    """Per-core Bass program for T padded tokens (T % 512 == 0); core e runs
    expert e."""
    TF = 512
    nc = Bacc()
    xT = nc.dram_tensor("xT", [P, KO_H, T], BF16, kind="ExternalInput")
    wg = nc.dram_tensor("wg", [P, KO_I, KO_H, P], BF16, kind="ExternalInput")
    wu = nc.dram_tensor("wu", [P, KO_I, KO_H, P], BF16, kind="ExternalInput")
    wd = nc.dram_tensor("wd", [P, KO_I, HID], BF16, kind="ExternalInput")
    probs = nc.dram_tensor("probs", [P, T // P], F32, kind="ExternalInput")
    out = nc.dram_tensor("out", [T, HID], F32, kind="ExternalOutput")

    n_tf = T // TF
    n_t = T // P
    n_nf = HID // NF

    with tile.TileContext(nc) as tc, ExitStack() as ctx:
        resident = ctx.enter_context(tc.tile_pool(name="resident", bufs=1))
        wpool = ctx.enter_context(tc.tile_pool(name="weights", bufs=2))
        tmp = ctx.enter_context(tc.tile_pool(name="tmp", bufs=3))
        opool = ctx.enter_context(tc.tile_pool(name="outp", bufs=4))
        psum = ctx.enter_context(tc.tile_pool(name="psum", bufs=2, space="PSUM"))

        xT_sb = resident.tile([P, KO_H, T], BF16)
        for k in range(KO_H):
            nc.sync.dma_start(xT_sb[:, k], xT[:, k])
        wd_sb = resident.tile([P, KO_I, HID], BF16)
        for k in range(KO_I):
            nc.sync.dma_start(wd_sb[:, k], wd[:, k])
        probs_dma = resident.tile([P, T // P], F32)
        nc.sync.dma_start(probs_dma[:], probs[:])
        probs_sb = resident.tile([P, T // P], F32)
        nc.vector.tensor_copy(probs_sb[:], probs_dma[:])
        hT_sb = resident.tile([P, KO_I, T], BF16)

        for m in range(KO_I):
            wg_m = wpool.tile([P, KO_H, P], BF16, tag="wg")
            nc.gpsimd.dma_start(wg_m[:], wg[:, m])
            wu_m = wpool.tile([P, KO_H, P], BF16, tag="wu")
            nc.gpsimd.dma_start(wu_m[:], wu[:, m])
            for f in range(n_tf):
                pg = psum.tile([P, TF], F32, tag="pg")
                pu = psum.tile([P, TF], F32, tag="pu")
                for k in range(KO_H):
                    nc.tensor.matmul(
                        pg[:], wg_m[:, k], xT_sb[:, k, bass.ts(f, TF)],
                        start=(k == 0), stop=(k == KO_H - 1),
                    )
                for k in range(KO_H):
                    nc.tensor.matmul(
                        pu[:], wu_m[:, k], xT_sb[:, k, bass.ts(f, TF)],
                        start=(k == 0), stop=(k == KO_H - 1),
                    )
                sg = tmp.tile([P, TF], F32, tag="sg")
                nc.scalar.activation(
                    sg[:], pg[:], mybir.ActivationFunctionType.Silu
                )
                su = tmp.tile([P, TF], F32, tag="su")
                nc.scalar.copy(su[:], pu[:])
                nc.vector.tensor_mul(
                    hT_sb[:, m, bass.ts(f, TF)], sg[:], su[:]
                )

        for t in range(n_t):
            for n in range(n_nf):
                po = psum.tile([P, NF], F32, tag="po")
                for k in range(KO_I):
                    nc.tensor.matmul(
                        po[:], hT_sb[:, k, bass.ts(t, P)],
                        wd_sb[:, k, bass.ts(n, NF)],
                        start=(k == 0), stop=(k == KO_I - 1),
                    )
                ot = opool.tile([P, NF], F32, tag="ot")
                nc.vector.tensor_scalar_mul(ot[:], po[:], probs_sb[:, t : t + 1])
                nc.sync.dma_start(out[bass.ts(t, P), bass.ts(n, NF)], ot[:])
    nc.finalize()
    return nc


def _run_padded(x, probs, wg, wu, wd, counts, offs, trace=False):
    T = int(max(1, counts.max()))
    T = ((T + 511) // 512) * 512

    key = ("padded", T)
    if key not in _nc_cache:
        _nc_cache[key] = _build_padded(T)
    nc = _nc_cache[key]

    in_maps = []
    for e in range(E):
        n = int(counts[e])
        s = int(offs[e])
        x_pad = np.zeros((T, HID), np.float32)
        x_pad[:n] = x[s : s + n]
        p_pad = np.zeros((T,), np.float32)
        p_pad[:n] = probs[s : s + n]
        xT = np.ascontiguousarray(
            x_pad.T.reshape(KO_H, P, T).transpose(1, 0, 2)
        ).astype(NP_BF16)
        wgt, wut, wdt = _pack_expert(wg[e], wu[e], wd[e])
        pr = np.ascontiguousarray(p_pad.reshape(T // P, P).T).astype(np.float32)
        in_maps.append({"xT": xT, "wg": wgt, "wu": wut, "wd": wdt, "probs": pr})

    res = run_bass_kernel_spmd(nc, in_maps, core_ids=list(range(E)), trace=trace)

    y = np.empty((x.shape[0], HID), np.float32)
    for e in range(E):
        n = int(counts[e])
        s = int(offs[e])
        y[s : s + n] = res.results[e]["out"][:n]
    return y, res


# -------------------------------------------------------------------- driver

def _run(inputs, trace=False):
    x = np.asarray(inputs["permuted_x"], np.float32)
    probs = np.asarray(inputs["permuted_probs"], np.float32)
    wg = np.asarray(inputs["w_gate"], np.float32)
    wu = np.asarray(inputs["w_up"], np.float32)
    wd = np.asarray(inputs["w_down"], np.float32)
    counts = np.asarray(inputs["tokens_per_expert"]).astype(np.int64)
    offs = np.concatenate([[0], np.cumsum(counts)])
    assert offs[-1] == x.shape[0]

    plan = None
    if x.shape[0] == E * T_CORE and len(counts) == E:
        plan = _plan([int(c) for c in counts])
    if plan is not None:
        return _run_packed(x, probs, wg, wu, wd, counts, offs, plan, trace=trace)
    return _run_padded(x, probs, wg, wu, wd, counts, offs, trace=trace)


def kernel(**inputs) -> np.ndarray:
    y, _ = _run(inputs, trace=False)
    return y


# revision 7
# speedup vs baseline: 1.2996x; 1.0749x over previous
"""GroupedSwiGLU MoE kernel for 8x Trainium2 NeuronCores.

Strategy: token-balanced packing. Every core gets exactly 1024 tokens as
three fixed-length segments (384, 384, 256); each segment is bound to one
expert's weights (passed as that slot's weight inputs). The given
tokens_per_expert decompose exactly into 16x384 + 8x256 chunks, so all
cores do identical FLOPs (the PE roofline) with zero padding.

Inside each core, per segment:
  phase 1: gateT/upT[inter, tok] psum chains contracting HID, then
           fused silu*mul into a resident hT buffer (bf16)
  phase 2: out[tok, hid] chains contracting INTER against the slot's
           w_down, scaled by probs
All matmul operands bf16 (fp32 PSUM accumulate); host does the
transpose/tiling and the final scatter back to original token order.

Falls back to the padded expert-parallel scheme (core e = expert e,
padded to the max group size) for inputs that don't fit the packing.
"""

import numpy as np
import ml_dtypes
from contextlib import ExitStack

import concourse.bass as bass
import concourse.mybir as mybir
import concourse.tile as tile
from concourse.bacc import Bacc
from concourse.bass_utils import run_bass_kernel_spmd

E = 8
HID = 2048
INTER = 1408
P = 128
KO_H = HID // P    # 16 k-tiles for phase-1 contraction
KO_I = INTER // P  # 11 k-tiles for phase-2 contraction / m-tiles in phase 1
NF = 512           # phase-2 moving free chunk (hid)

SEG_LENS = (384, 384, 256)  # per-core token segments (sum = 1024)
T_CORE = sum(SEG_LENS)

F32 = mybir.dt.float32
BF16 = mybir.dt.bfloat16
NP_BF16 = ml_dtypes.bfloat16

_nc_cache: dict = {}


# ---------------------------------------------------------------- packed path

def _build_packed():
    """Per-core Bass program: 1024 tokens in segments SEG_LENS, one expert
    weight set per segment slot."""
    nc = Bacc()
    xs = [
        nc.dram_tensor(f"x{j}", [P, KO_H, L], BF16, kind="ExternalInput")
        for j, L in enumerate(SEG_LENS)
    ]
    wgs = [
        nc.dram_tensor(f"wg{j}", [P, KO_I, KO_H, P], BF16, kind="ExternalInput")
        for j in range(len(SEG_LENS))
    ]
    wus = [
        nc.dram_tensor(f"wu{j}", [P, KO_I, KO_H, P], BF16, kind="ExternalInput")
        for j in range(len(SEG_LENS))
    ]
    wds = [
        nc.dram_tensor(f"wd{j}", [P, KO_I, HID], BF16, kind="ExternalInput")
        for j in range(len(SEG_LENS))
    ]
    probs = nc.dram_tensor("probs", [P, T_CORE // P], F32, kind="ExternalInput")
    out = nc.dram_tensor("out", [T_CORE, HID], F32, kind="ExternalOutput")

    lmax = max(SEG_LENS)

    with tile.TileContext(nc) as tc, ExitStack() as ctx:
        resident = ctx.enter_context(tc.tile_pool(name="resident", bufs=1))
        wpool = ctx.enter_context(tc.tile_pool(name="weights", bufs=2))
        wdpool = ctx.enter_context(tc.tile_pool(name="wdp", bufs=2))
        tmp = ctx.enter_context(tc.tile_pool(name="tmp", bufs=3))
        opool = ctx.enter_context(tc.tile_pool(name="outp", bufs=3))
        psum = ctx.enter_context(tc.tile_pool(name="psum", bufs=2, space="PSUM"))

        # x segment loads in 4-k chunks so the first chain only waits on a
        # 0.4MB transfer; segments 1/2 are deferred to earlier phase-2 spans.
        def load_x(j, xt):
            for k0 in range(0, KO_H, 4):
                nc.sync.dma_start(xt[:, k0 : k0 + 4], xs[j][:, k0 : k0 + 4])

        x_sb = [
            resident.tile([P, KO_H, L], BF16, name=f"x_sb{j}")
            for j, L in enumerate(SEG_LENS)
        ]
        load_x(0, x_sb[0])
        probs_dma = resident.tile([P, T_CORE // P], F32)
        nc.sync.dma_start(probs_dma[:], probs[:])
        # Bounce through DVE so phase-2 scaling (DVE) only ever needs the PE
        # wait: the TensorScalar ISA slot can't carry a second (DMA) wait.
        probs_sb = resident.tile([P, T_CORE // P], F32)
        nc.vector.tensor_copy(probs_sb[:], probs_dma[:])
        hT_sb = resident.tile([P, KO_I, T_CORE], BF16)

        t0 = 0
        for j, L in enumerate(SEG_LENS):
            # w_down for this segment: per-k chunks issued from the scalar
            # (ACT) queue, paced through ph1 by emitting behind m-loop work so
            # the big transfer never starves the wg/wu stream or x loads.
            wd_sb = wdpool.tile([P, KO_I, HID], BF16, tag="wd")

            # Phase 1: per inter m-tile, gateT/upT psum then fused silu*mul
            for m in range(KO_I):
                wg_m = wpool.tile([P, KO_H, P], BF16, tag="wg")
                nc.gpsimd.dma_start(wg_m[:], wgs[j][:, m])
                wu_m = wpool.tile([P, KO_H, P], BF16, tag="wu")
                nc.gpsimd.dma_start(wu_m[:], wus[j][:, m])
                if m >= 1:
                    # 11 wd k-chunks spread over m = 1..10, one per m-loop
                    # (last m emits two so all KO_I chunks are issued)
                    for k in [m - 1] + ([KO_I - 1] if m == KO_I - 1 else []):
                        nc.scalar.dma_start(wd_sb[:, k], wds[j][:, k])
                pg = psum.tile([P, lmax], F32, tag="pg")
                pu = psum.tile([P, lmax], F32, tag="pu")
                for k in range(KO_H):
                    nc.tensor.matmul(
                        pg[:, :L], wg_m[:, k], x_sb[j][:, k],
                        start=(k == 0), stop=(k == KO_H - 1),
                    )
                for k in range(KO_H):
                    nc.tensor.matmul(
                        pu[:, :L], wu_m[:, k], x_sb[j][:, k],
                        start=(k == 0), stop=(k == KO_H - 1),
                    )
                sg = tmp.tile([P, lmax], F32, tag="sg")
                nc.scalar.activation(
                    sg[:, :L], pg[:, :L], mybir.ActivationFunctionType.Silu
                )
                # ACT copy of up-psum so the DVE mul has a single-engine wait
                su = tmp.tile([P, lmax], F32, tag="su")
                nc.scalar.copy(su[:, :L], pu[:, :L])
                nc.vector.tensor_mul(
                    hT_sb[:, m, t0 : t0 + L], sg[:, :L], su[:, :L]
                )

            # Prefetch the next segment's x while this segment's ph2 runs
            if j + 1 < len(SEG_LENS):
                load_x(j + 1, x_sb[j + 1])

            # Phase 2: out tiles [128 tok, 2048 hid], contract over inter
            for tt in range(t0 // P, (t0 + L) // P):
                ot = opool.tile([P, HID], F32, tag="ot")
                for n in range(HID // NF):
                    po = psum.tile([P, NF], F32, tag="po")
                    for k in range(KO_I):
                        nc.tensor.matmul(
                            po[:], hT_sb[:, k, tt * P : (tt + 1) * P],
                            wd_sb[:, k, n * NF : (n + 1) * NF],
                            start=(k == 0), stop=(k == KO_I - 1),
                        )
                    nc.vector.tensor_scalar_mul(
                        ot[:, n * NF : (n + 1) * NF], po[:],
                        probs_sb[:, tt : tt + 1],
                    )
                nc.sync.dma_start(out[tt * P : (tt + 1) * P, :], ot[:])
            t0 += L
    nc.finalize()
    return nc


def _plan(counts):
    """Decompose expert token counts into 16x384 + 8x256 chunks and assign
    them to 8 cores x slots (384, 384, 256). Returns per-core slot lists
    [(expert, start_within_expert, length), ...] or None if infeasible."""
    n384 = 2 * E
    n256 = E
    opts_per_e = []
    for c in counts:
        opts = []
        for a in range(c // 384 + 1):
            r = c - 384 * a
            if r >= 0 and r % 256 == 0:
                opts.append((a, r // 256))
        if not opts:
            return None
        opts_per_e.append(opts)

    def rec(i, a, b, acc):
        if i == len(counts):
            return list(acc) if (a == n384 and b == n256) else None
        for ae, be in opts_per_e[i]:
            if a + ae <= n384 and b + be <= n256:
                acc.append((ae, be))
                r = rec(i + 1, a + ae, b + be, acc)
                if r is not None:
                    return r
                acc.pop()
        return None

    decomp = rec(0, 0, 0, [])
    if decomp is None:
        return None

    chunks384, chunks256 = [], []
    for e, (a, b) in enumerate(decomp):
        for i in range(a):
            chunks384.append((e, i * 384, 384))
        for i in range(b):
            chunks256.append((e, a * 384 + i * 256, 256))
    return [
        [chunks384[2 * c], chunks384[2 * c + 1], chunks256[c]] for c in range(E)
    ]


def _pack_expert(wg_e, wu_e, wd_e):
    """Tile one expert's weights into the DRAM layouts the kernel expects."""
    # wg[p, m, k, i] = w_gate[k*128+p, m*128+i]
    wgt = np.ascontiguousarray(
        wg_e.reshape(KO_H, P, KO_I, P).transpose(1, 2, 0, 3)
    ).astype(NP_BF16)
    wut = np.ascontiguousarray(
        wu_e.reshape(KO_H, P, KO_I, P).transpose(1, 2, 0, 3)
    ).astype(NP_BF16)
    # wd[p, k, h] = w_down[k*128+p, h]
    wdt = np.ascontiguousarray(
        wd_e.reshape(KO_I, P, HID).transpose(1, 0, 2)
    ).astype(NP_BF16)
    return wgt, wut, wdt


def _pack_xT(x_rows):
    """xT[p, k, t] = x_rows[t, k*128+p] for one segment."""
    L = x_rows.shape[0]
    return np.ascontiguousarray(
        x_rows.T.reshape(KO_H, P, L).transpose(1, 0, 2)
    ).astype(NP_BF16)


def _run_packed(x, probs, wg, wu, wd, counts, offs, plan, trace=False):
    if "packed" not in _nc_cache:
        _nc_cache["packed"] = _build_packed()
    nc = _nc_cache["packed"]

    packed_w = [_pack_expert(wg[e], wu[e], wd[e]) for e in range(E)]

    in_maps = []
    for c in range(E):
        m = {}
        p_core = np.empty((T_CORE,), np.float32)
        t0 = 0
        for j, (e, s, L) in enumerate(plan[c]):
            g = int(offs[e]) + s
            m[f"x{j}"] = _pack_xT(x[g : g + L])
            m[f"wg{j}"], m[f"wu{j}"], m[f"wd{j}"] = packed_w[e]
            p_core[t0 : t0 + L] = probs[g : g + L]
            t0 += L
        m["probs"] = np.ascontiguousarray(
            p_core.reshape(T_CORE // P, P).T
        ).astype(np.float32)
        in_maps.append(m)

    res = run_bass_kernel_spmd(nc, in_maps, core_ids=list(range(E)), trace=trace)

    y = np.empty((x.shape[0], HID), np.float32)
    for c in range(E):
        o = res.results[c]["out"]
        t0 = 0
        for e, s, L in plan[c]:
            g = int(offs[e]) + s
            y[g : g + L] = o[t0 : t0 + L]
            t0 += L
    return y, res


# ------------------------------------------------- padded fallback (generic)

def _build_padded(T: int):
    """Per-core Bass program for T padded tokens (T % 512 == 0); core e runs
    expert e."""
    TF = 512
    nc = Bacc()
    xT = nc.dram_tensor("xT", [P, KO_H, T], BF16, kind="ExternalInput")
    wg = nc.dram_tensor("wg", [P, KO_I, KO_H, P], BF16, kind="ExternalInput")
    wu = nc.dram_tensor("wu", [P, KO_I, KO_H, P], BF16, kind="ExternalInput")
    wd = nc.dram_tensor("wd", [P, KO_I, HID], BF16, kind="ExternalInput")
    probs = nc.dram_tensor("probs", [P, T // P], F32, kind="ExternalInput")
    out = nc.dram_tensor("out", [T, HID], F32, kind="ExternalOutput")

    n_tf = T // TF
    n_t = T // P
    n_nf = HID // NF

    with tile.TileContext(nc) as tc, ExitStack() as ctx:
        resident = ctx.enter_context(tc.tile_pool(name="resident", bufs=1))
        wpool = ctx.enter_context(tc.tile_pool(name="weights", bufs=2))
        tmp = ctx.enter_context(tc.tile_pool(name="tmp", bufs=3))
        opool = ctx.enter_context(tc.tile_pool(name="outp", bufs=4))
        psum = ctx.enter_context(tc.tile_pool(name="psum", bufs=2, space="PSUM"))

        xT_sb = resident.tile([P, KO_H, T], BF16)
        for k in range(KO_H):
            nc.sync.dma_start(xT_sb[:, k], xT[:, k])
        wd_sb = resident.tile([P, KO_I, HID], BF16)
        for k in range(KO_I):
            nc.sync.dma_start(wd_sb[:, k], wd[:, k])
        probs_dma = resident.tile([P, T // P], F32)
        nc.sync.dma_start(probs_dma[:], probs[:])
        probs_sb = resident.tile([P, T // P], F32)
        nc.vector.tensor_copy(probs_sb[:], probs_dma[:])
        hT_sb = resident.tile([P, KO_I, T], BF16)

        for m in range(KO_I):
            wg_m = wpool.tile([P, KO_H, P], BF16, tag="wg")
            nc.gpsimd.dma_start(wg_m[:], wg[:, m])
            wu_m = wpool.tile([P, KO_H, P], BF16, tag="wu")
            nc.gpsimd.dma_start(wu_m[:], wu[:, m])
            for f in range(n_tf):
                pg = psum.tile([P, TF], F32, tag="pg")
                pu = psum.tile([P, TF], F32, tag="pu")
                for k in range(KO_H):
                    nc.tensor.matmul(
                        pg[:], wg_m[:, k], xT_sb[:, k, bass.ts(f, TF)],
                        start=(k == 0), stop=(k == KO_H - 1),
                    )
                for k in range(KO_H):
                    nc.tensor.matmul(
                        pu[:], wu_m[:, k], xT_sb[:, k, bass.ts(f, TF)],
                        start=(k == 0), stop=(k == KO_H - 1),
                    )
                sg = tmp.tile([P, TF], F32, tag="sg")
                nc.scalar.activation(
                    sg[:], pg[:], mybir.ActivationFunctionType.Silu
                )
                su = tmp.tile([P, TF], F32, tag="su")
                nc.scalar.copy(su[:], pu[:])
                nc.vector.tensor_mul(
                    hT_sb[:, m, bass.ts(f, TF)], sg[:], su[:]
                )

        for t in range(n_t):
            for n in range(n_nf):
                po = psum.tile([P, NF], F32, tag="po")
                for k in range(KO_I):
                    nc.tensor.matmul(
                        po[:], hT_sb[:, k, bass.ts(t, P)],
                        wd_sb[:, k, bass.ts(n, NF)],
                        start=(k == 0), stop=(k == KO_I - 1),
                    )
                ot = opool.tile([P, NF], F32, tag="ot")
                nc.vector.tensor_scalar_mul(ot[:], po[:], probs_sb[:, t : t + 1])
                nc.sync.dma_start(out[bass.ts(t, P), bass.ts(n, NF)], ot[:])
    nc.finalize()
    return nc


def _run_padded(x, probs, wg, wu, wd, counts, offs, trace=False):
    T = int(max(1, counts.max()))
    T = ((T + 511) // 512) * 512

    key = ("padded", T)
    if key not in _nc_cache:
        _nc_cache[key] = _build_padded(T)
    nc = _nc_cache[key]

    in_maps = []
    for e in range(E):
        n = int(counts[e])
        s = int(offs[e])
        x_pad = np.zeros((T, HID), np.float32)
        x_pad[:n] = x[s : s + n]
        p_pad = np.zeros((T,), np.float32)
        p_pad[:n] = probs[s : s + n]
        xT = np.ascontiguousarray(
            x_pad.T.reshape(KO_H, P, T).transpose(1, 0, 2)
        ).astype(NP_BF16)
        wgt, wut, wdt = _pack_expert(wg[e], wu[e], wd[e])
        pr = np.ascontiguousarray(p_pad.reshape(T // P, P).T).astype(np.float32)
        in_maps.append({"xT": xT, "wg": wgt, "wu": wut, "wd": wdt, "probs": pr})

    res = run_bass_kernel_spmd(nc, in_maps, core_ids=list(range(E)), trace=trace)

    y = np.empty((x.shape[0], HID), np.float32)
    for e in range(E):
        n = int(counts[e])
        s = int(offs[e])
        y[s : s + n] = res.results[e]["out"][:n]
    return y, res


# -------------------------------------------------------------------- driver

def _run(inputs, trace=False):
    x = np.asarray(inputs["permuted_x"], np.float32)
    probs = np.asarray(inputs["permuted_probs"], np.float32)
    wg = np.asarray(inputs["w_gate"], np.float32)
    wu = np.asarray(inputs["w_up"], np.float32)
    wd = np.asarray(inputs["w_down"], np.float32)
    counts = np.asarray(inputs["tokens_per_expert"]).astype(np.int64)
    offs = np.concatenate([[0], np.cumsum(counts)])
    assert offs[-1] == x.shape[0]

    plan = None
    if x.shape[0] == E * T_CORE and len(counts) == E:
        plan = _plan([int(c) for c in counts])
    if plan is not None:
        return _run_packed(x, probs, wg, wu, wd, counts, offs, plan, trace=trace)
    return _run_padded(x, probs, wg, wu, wd, counts, offs, trace=trace)


def kernel(**inputs) -> np.ndarray:
    y, _ = _run(inputs, trace=False)
    return y


# revision 13
# speedup vs baseline: 1.4642x; 1.1266x over previous
"""GroupedSwiGLU MoE kernel for 8x Trainium2 NeuronCores.

Strategy: token-balanced packing. Every core gets exactly 1024 tokens as
three fixed-length segments (384, 384, 256); each segment is bound to one
expert's weights (passed as that slot's weight inputs). The given
tokens_per_expert decompose exactly into 16x384 + 8x256 chunks, so all
cores do identical FLOPs (the PE roofline) with zero padding.

Inside each core, per segment:
  phase 1: gateT/upT[inter, tok] psum chains contracting HID, then
           fused silu*mul into a resident hT buffer (bf16)
  phase 2: out[tok, hid] chains contracting INTER against the slot's
           w_down, scaled by probs
All matmul operands bf16 (fp32 PSUM accumulate); host does the
transpose/tiling and the final scatter back to original token order.

Falls back to the padded expert-parallel scheme (core e = expert e,
padded to the max group size) for inputs that don't fit the packing.
"""

import numpy as np
import ml_dtypes
from contextlib import ExitStack

import concourse.bass as bass
import concourse.mybir as mybir
import concourse.tile as tile
from concourse.bacc import Bacc
from concourse.bass_utils import run_bass_kernel_spmd

E = 8
HID = 2048
INTER = 1408
P = 128
KO_H = HID // P    # 16 k-tiles for phase-1 contraction
KO_I = INTER // P  # 11 k-tiles for phase-2 contraction / m-tiles in phase 1
NF = 512           # phase-2 moving free chunk (hid)

SEG_LENS = (384, 384, 256)  # per-core token segments (sum = 1024)
T_CORE = sum(SEG_LENS)

F32 = mybir.dt.float32
BF16 = mybir.dt.bfloat16
NP_BF16 = ml_dtypes.bfloat16

_nc_cache: dict = {}


# ---------------------------------------------------------------- packed path

def _build_packed():
    """Per-core Bass program: 1024 tokens in segments SEG_LENS, one expert
    weight set per segment slot."""
    nc = Bacc()
    xs = [
        nc.dram_tensor(f"x{j}", [P, KO_H, L], BF16, kind="ExternalInput")
        for j, L in enumerate(SEG_LENS)
    ]
    wgs = [
        nc.dram_tensor(f"wg{j}", [P, KO_I, KO_H, P], BF16, kind="ExternalInput")
        for j in range(len(SEG_LENS))
    ]
    wus = [
        nc.dram_tensor(f"wu{j}", [P, KO_I, KO_H, P], BF16, kind="ExternalInput")
        for j in range(len(SEG_LENS))
    ]
    wds = [
        nc.dram_tensor(f"wd{j}", [P, KO_I, HID], BF16, kind="ExternalInput")
        for j in range(len(SEG_LENS))
    ]
    probs = nc.dram_tensor("probs", [P, T_CORE // P], F32, kind="ExternalInput")
    out = nc.dram_tensor("out", [T_CORE, HID], F32, kind="ExternalOutput")

    lmax = max(SEG_LENS)

    with tile.TileContext(nc) as tc, ExitStack() as ctx:
        resident = ctx.enter_context(tc.tile_pool(name="resident", bufs=1))
        wpool = ctx.enter_context(tc.tile_pool(name="weights", bufs=3))
        wdpool = ctx.enter_context(tc.tile_pool(name="wdp", bufs=2))
        tmp = ctx.enter_context(tc.tile_pool(name="tmp", bufs=3))
        opool = ctx.enter_context(tc.tile_pool(name="outp", bufs=2))
        psum = ctx.enter_context(tc.tile_pool(name="psum", bufs=2, space="PSUM"))

        # x segment loads in 4-k chunks so the first chain only waits on a
        # 0.4MB transfer; segments 1/2 are deferred to earlier phase-2 spans.
        def load_x(j, xt):
            for k0 in range(0, KO_H, 4):
                nc.sync.dma_start(xt[:, k0 : k0 + 4], xs[j][:, k0 : k0 + 4])

        x_sb = [
            resident.tile([P, KO_H, L], BF16, name=f"x_sb{j}")
            for j, L in enumerate(SEG_LENS)
        ]
        load_x(0, x_sb[0])
        probs_dma = resident.tile([P, T_CORE // P], F32)
        nc.sync.dma_start(probs_dma[:], probs[:])
        probs_sb = resident.tile([P, T_CORE // P], F32)
        hT_sb = resident.tile([P, KO_I, T_CORE], BF16)

        # All weight streams ride the scalar (ACT) HWDGE queue. Configs are
        # emitted two m-loops ahead of use so silu's PE-wait (which blocks
        # the in-order ACT sequencer) never delays the next weight DMA.
        ms = [(j, m) for j in range(len(SEG_LENS)) for m in range(KO_I)]
        wtiles = {}

        def cfg_weights(idx):
            if idx >= len(ms):
                return
            jj, mm = ms[idx]
            wg_m = wpool.tile([P, KO_H, P], BF16, tag="wg")
            nc.scalar.dma_start(wg_m[:], wgs[jj][:, mm])
            wu_m = wpool.tile([P, KO_H, P], BF16, tag="wu")
            nc.scalar.dma_start(wu_m[:], wus[jj][:, mm])
            wtiles[(jj, mm)] = (wg_m, wu_m)

        cfg_weights(0)
        cfg_weights(1)

        t0 = 0
        mi = 0
        for j, L in enumerate(SEG_LENS):
            # w_down for this segment: per-k chunks paced through ph1 behind
            # the silu/copy stream so the big transfer never starves wg/wu.
            wd_sb = wdpool.tile([P, KO_I, HID], BF16, tag="wd")

            # Phase 1: per inter m-tile, gateT/upT psum then fused silu*mul
            for m in range(KO_I):
                cfg_weights(mi + 2)
                mi += 1
                if m >= 1:
                    # 11 wd k-chunks spread over m = 1..10, one per m-loop
                    # (last m emits two so all KO_I chunks are issued)
                    for k in [m - 1] + ([KO_I - 1] if m == KO_I - 1 else []):
                        nc.scalar.dma_start(wd_sb[:, k], wds[j][:, k])
                wg_m, wu_m = wtiles.pop((j, m))
                pg = psum.tile([P, lmax], F32, tag="pg")
                pu = psum.tile([P, lmax], F32, tag="pu")
                for k in range(KO_H):
                    nc.tensor.matmul(
                        pg[:, :L], wg_m[:, k], x_sb[j][:, k],
                        start=(k == 0), stop=(k == KO_H - 1),
                    )
                for k in range(KO_H):
                    nc.tensor.matmul(
                        pu[:, :L], wu_m[:, k], x_sb[j][:, k],
                        start=(k == 0), stop=(k == KO_H - 1),
                    )
                sg = tmp.tile([P, lmax], F32, tag="sg")
                nc.scalar.activation(
                    sg[:, :L], pg[:, :L], mybir.ActivationFunctionType.Silu
                )
                # ACT copy of up-psum so the DVE mul has a single-engine wait
                su = tmp.tile([P, lmax], F32, tag="su")
                nc.scalar.copy(su[:, :L], pu[:, :L])
                nc.vector.tensor_mul(
                    hT_sb[:, m, t0 : t0 + L], sg[:, :L], su[:, :L]
                )

            # Prefetch the next segment's x while this segment's ph2 runs
            if j + 1 < len(SEG_LENS):
                load_x(j + 1, x_sb[j + 1])
            if j == 0:
                # Bounce probs through DVE so phase-2 scaling (DVE) only ever
                # needs the PE wait: the TensorScalar ISA slot can't carry a
                # second (DMA) wait. Emitted here so it doesn't block the DVE
                # queue's wg/wu prefetch at startup.
                nc.vector.tensor_copy(probs_sb[:], probs_dma[:])

            # Phase 2: out tiles [128 tok, 2048 hid], contract over inter
            last_seg = j == len(SEG_LENS) - 1
            for tt in range(t0 // P, (t0 + L) // P):
                last_tt = last_seg and tt == (t0 + L) // P - 1
                ot = opool.tile([P, HID], F32, tag="ot")
                for n in range(HID // NF):
                    po = psum.tile([P, NF], F32, tag="po")
                    for k in range(KO_I):
                        nc.tensor.matmul(
                            po[:], hT_sb[:, k, tt * P : (tt + 1) * P],
                            wd_sb[:, k, n * NF : (n + 1) * NF],
                            start=(k == 0), stop=(k == KO_I - 1),
                        )
                    nc.vector.tensor_scalar_mul(
                        ot[:, n * NF : (n + 1) * NF], po[:],
                        probs_sb[:, tt : tt + 1],
                    )
                    if last_tt:
                        # stream the final tile per n-chunk so the last DMA
                        # overlaps the remaining chains instead of tailing
                        nc.sync.dma_start(
                            out[tt * P : (tt + 1) * P, n * NF : (n + 1) * NF],
                            ot[:, n * NF : (n + 1) * NF],
                        )
                if not last_tt:
                    nc.sync.dma_start(out[tt * P : (tt + 1) * P, :], ot[:])
            t0 += L
    nc.finalize()
    return nc


def _plan(counts):
    """Decompose expert token counts into 16x384 + 8x256 chunks and assign
    them to 8 cores x slots (384, 384, 256). Returns per-core slot lists
    [(expert, start_within_expert, length), ...] or None if infeasible."""
    n384 = 2 * E
    n256 = E
    opts_per_e = []
    for c in counts:
        opts = []
        for a in range(c // 384 + 1):
            r = c - 384 * a
            if r >= 0 and r % 256 == 0:
                opts.append((a, r // 256))
        if not opts:
            return None
        opts_per_e.append(opts)

    def rec(i, a, b, acc):
        if i == len(counts):
            return list(acc) if (a == n384 and b == n256) else None
        for ae, be in opts_per_e[i]:
            if a + ae <= n384 and b + be <= n256:
                acc.append((ae, be))
                r = rec(i + 1, a + ae, b + be, acc)
                if r is not None:
                    return r
                acc.pop()
        return None

    decomp = rec(0, 0, 0, [])
    if decomp is None:
        return None

    chunks384, chunks256 = [], []
    for e, (a, b) in enumerate(decomp):
        for i in range(a):
            chunks384.append((e, i * 384, 384))
        for i in range(b):
            chunks256.append((e, a * 384 + i * 256, 256))
    return [
        [chunks384[2 * c], chunks384[2 * c + 1], chunks256[c]] for c in range(E)
    ]


def _pack_expert(wg_e, wu_e, wd_e):
    """Tile one expert's weights into the DRAM layouts the kernel expects."""
    # wg[p, m, k, i] = w_gate[k*128+p, m*128+i]
    wgt = np.ascontiguousarray(
        wg_e.reshape(KO_H, P, KO_I, P).transpose(1, 2, 0, 3)
    ).astype(NP_BF16)
    wut = np.ascontiguousarray(
        wu_e.reshape(KO_H, P, KO_I, P).transpose(1, 2, 0, 3)
    ).astype(NP_BF16)
    # wd[p, k, h] = w_down[k*128+p, h]
    wdt = np.ascontiguousarray(
        wd_e.reshape(KO_I, P, HID).transpose(1, 0, 2)
    ).astype(NP_BF16)
    return wgt, wut, wdt


def _pack_xT(x_rows):
    """xT[p, k, t] = x_rows[t, k*128+p] for one segment."""
    L = x_rows.shape[0]
    return np.ascontiguousarray(
        x_rows.T.reshape(KO_H, P, L).transpose(1, 0, 2)
    ).astype(NP_BF16)


def _run_packed(x, probs, wg, wu, wd, counts, offs, plan, trace=False):
    if "packed" not in _nc_cache:
        _nc_cache["packed"] = _build_packed()
    nc = _nc_cache["packed"]

    packed_w = [_pack_expert(wg[e], wu[e], wd[e]) for e in range(E)]

    in_maps = []
    for c in range(E):
        m = {}
        p_core = np.empty((T_CORE,), np.float32)
        t0 = 0
        for j, (e, s, L) in enumerate(plan[c]):
            g = int(offs[e]) + s
            m[f"x{j}"] = _pack_xT(x[g : g + L])
            m[f"wg{j}"], m[f"wu{j}"], m[f"wd{j}"] = packed_w[e]
            p_core[t0 : t0 + L] = probs[g : g + L]
            t0 += L
        m["probs"] = np.ascontiguousarray(
            p_core.reshape(T_CORE // P, P).T
        ).astype(np.float32)
        in_maps.append(m)

    res = run_bass_kernel_spmd(nc, in_maps, core_ids=list(range(E)), trace=trace)

    y = np.empty((x.shape[0], HID), np.float32)
    for c in range(E):
        o = res.results[c]["out"]
        t0 = 0
        for e, s, L in plan[c]:
            g = int(offs[e]) + s
            y[g : g + L] = o[t0 : t0 + L]
            t0 += L
    return y, res


# ------------------------------------------------- padded fallback (generic)

def _build_padded(T: int):
    """Per-core Bass program for T padded tokens (T % 512 == 0); core e runs
    expert e."""
    TF = 512
    nc = Bacc()
    xT = nc.dram_tensor("xT", [P, KO_H, T], BF16, kind="ExternalInput")
    wg = nc.dram_tensor("wg", [P, KO_I, KO_H, P], BF16, kind="ExternalInput")
    wu = nc.dram_tensor("wu", [P, KO_I, KO_H, P], BF16, kind="ExternalInput")
    wd = nc.dram_tensor("wd", [P, KO_I, HID], BF16, kind="ExternalInput")
    probs = nc.dram_tensor("probs", [P, T // P], F32, kind="ExternalInput")
    out = nc.dram_tensor("out", [T, HID], F32, kind="ExternalOutput")

    n_tf = T // TF
    n_t = T // P
    n_nf = HID // NF

    with tile.TileContext(nc) as tc, ExitStack() as ctx:
        resident = ctx.enter_context(tc.tile_pool(name="resident", bufs=1))
        wpool = ctx.enter_context(tc.tile_pool(name="weights", bufs=2))
        tmp = ctx.enter_context(tc.tile_pool(name="tmp", bufs=3))
        opool = ctx.enter_context(tc.tile_pool(name="outp", bufs=4))
        psum = ctx.enter_context(tc.tile_pool(name="psum", bufs=2, space="PSUM"))

        xT_sb = resident.tile([P, KO_H, T], BF16)
        for k in range(KO_H):
            nc.sync.dma_start(xT_sb[:, k], xT[:, k])
        wd_sb = resident.tile([P, KO_I, HID], BF16)
        for k in range(KO_I):
            nc.sync.dma_start(wd_sb[:, k], wd[:, k])
        probs_dma = resident.tile([P, T // P], F32)
        nc.sync.dma_start(probs_dma[:], probs[:])
        probs_sb = resident.tile([P, T // P], F32)
        nc.vector.tensor_copy(probs_sb[:], probs_dma[:])
        hT_sb = resident.tile([P, KO_I, T], BF16)

        for m in range(KO_I):
            wg_m = wpool.tile([P, KO_H, P], BF16, tag="wg")
            nc.gpsimd.dma_start(wg_m[:], wg[:, m])
            wu_m = wpool.tile([P, KO_H, P], BF16, tag="wu")
            nc.gpsimd.dma_start(wu_m[:], wu[:, m])
            for f in range(n_tf):
                pg = psum.tile([P, TF], F32, tag="pg")
                pu = psum.tile([P, TF], F32, tag="pu")
                for k in range(KO_H):
                    nc.tensor.matmul(
                        pg[:], wg_m[:, k], xT_sb[:, k, bass.ts(f, TF)],
                        start=(k == 0), stop=(k == KO_H - 1),
                    )
                for k in range(KO_H):
                    nc.tensor.matmul(
                        pu[:], wu_m[:, k], xT_sb[:, k, bass.ts(f, TF)],
                        start=(k == 0), stop=(k == KO_H - 1),
                    )
                sg = tmp.tile([P, TF], F32, tag="sg")
                nc.scalar.activation(
                    sg[:], pg[:], mybir.ActivationFunctionType.Silu
                )
                su = tmp.tile([P, TF], F32, tag="su")
                nc.scalar.copy(su[:], pu[:])
                nc.vector.tensor_mul(
                    hT_sb[:, m, bass.ts(f, TF)], sg[:], su[:]
                )

        for t in range(n_t):
            for n in range(n_nf):
                po = psum.tile([P, NF], F32, tag="po")
                for k in range(KO_I):
                    nc.tensor.matmul(
                        po[:], hT_sb[:, k, bass.ts(t, P)],
                        wd_sb[:, k, bass.ts(n, NF)],
                        start=(k == 0), stop=(k == KO_I - 1),
                    )
                ot = opool.tile([P, NF], F32, tag="ot")
                nc.vector.tensor_scalar_mul(ot[:], po[:], probs_sb[:, t : t + 1])
                nc.sync.dma_start(out[bass.ts(t, P), bass.ts(n, NF)], ot[:])
    nc.finalize()
    return nc


def _run_padded(x, probs, wg, wu, wd, counts, offs, trace=False):
    T = int(max(1, counts.max()))
    T = ((T + 511) // 512) * 512

    key = ("padded", T)
    if key not in _nc_cache:
        _nc_cache[key] = _build_padded(T)
    nc = _nc_cache[key]

    in_maps = []
    for e in range(E):
        n = int(counts[e])
        s = int(offs[e])
        x_pad = np.zeros((T, HID), np.float32)
        x_pad[:n] = x[s : s + n]
        p_pad = np.zeros((T,), np.float32)
        p_pad[:n] = probs[s : s + n]
        xT = np.ascontiguousarray(
            x_pad.T.reshape(KO_H, P, T).transpose(1, 0, 2)
        ).astype(NP_BF16)
        wgt, wut, wdt = _pack_expert(wg[e], wu[e], wd[e])
        pr = np.ascontiguousarray(p_pad.reshape(T // P, P).T).astype(np.float32)
        in_maps.append({"xT": xT, "wg": wgt, "wu": wut, "wd": wdt, "probs": pr})

    res = run_bass_kernel_spmd(nc, in_maps, core_ids=list(range(E)), trace=trace)

    y = np.empty((x.shape[0], HID), np.float32)
    for e in range(E):
        n = int(counts[e])
        s = int(offs[e])
        y[s : s + n] = res.results[e]["out"][:n]
    return y, res


# -------------------------------------------------------------------- driver

def _run(inputs, trace=False):
    x = np.asarray(inputs["permuted_x"], np.float32)
    probs = np.asarray(inputs["permuted_probs"], np.float32)
    wg = np.asarray(inputs["w_gate"], np.float32)
    wu = np.asarray(inputs["w_up"], np.float32)
    wd = np.asarray(inputs["w_down"], np.float32)
    counts = np.asarray(inputs["tokens_per_expert"]).astype(np.int64)
    offs = np.concatenate([[0], np.cumsum(counts)])
    assert offs[-1] == x.shape[0]

    plan = None
    if x.shape[0] == E * T_CORE and len(counts) == E:
        plan = _plan([int(c) for c in counts])
    if plan is not None:
        return _run_packed(x, probs, wg, wu, wd, counts, offs, plan, trace=trace)
    return _run_padded(x, probs, wg, wu, wd, counts, offs, trace=trace)


def kernel(**inputs) -> np.ndarray:
    y, _ = _run(inputs, trace=False)
    return y
